# revision 11
# baseline (speedup 1.0000x reference)
"""BayesNAM forward (dense per-feature MLP ensemble) on 8 Trainium2 NeuronCores.

Math (per batch row b, feature f; H=64 hidden):
    A0 = leaky(x[b,f] * W0e[f,:] + b0e[f,:])          W0e = wmu0 + exp(wls0)*ew0 ...
    A1 = leaky(A0 @ W1e[f] + b1e[f])
    y[b] = sum_f (A1 @ W2e[f] + b2e[f]) + bias
    kl   = scalar function of (mu, log_sigma) params only

Sharding: batch (16384) split across 8 cores (2048 each); parameters replicated.

Per-core kernel (fp16 matmul inputs, fp32 accumulation):
  X_T   [65, 2048] fp16  : transposed input; row 64 = ones (layer-0 bias row)
  T0all [65, 4096] fp16  : layer-0 weights; feature f's row diag-placed at cols
                           [64f, 64f+64) so one K=65 matmul computes a feature
                           PAIR's layer-0 (+bias); built via a DRAM bounce
  W1BLK [128, 4096] fp16 : layer-1 weights as block-diagonal [128,128] tiles
  W1T   [128, 4096] fp16 : PE-transposed W1BLK blocks, for V1 = 0.01*W1@w2
  pair loop (layer-2 emitted one pair behind to keep the in-order PE fed):
    l0 matmul (K=65, bias via ones row) -> pass1 ACT Lrelu [128,1024] ->
    l1 blockdiag matmul -> pass2: ACT Lrelu(+bias) for a few pairs, else
    DVE relu(z+b1) with the 0.01-linear leak restored by V1^T@A0 matmuls
    and a host constant -> M=1 matmuls accumulate y into one PSUM bank,
    column-packed 4-way by chunk.
  KL sums ride DVE (squares of the already-computed exp tiles); reduction
  matmuls run after the loop. y partial rows and constants combined on host.
"""
import sys
import numpy as np

for _p in ("/opt/trn_rl_repo", "/root/.axon_site/_ro/trn_rl_repo"):
    if _p not in sys.path:
        sys.path.append(_p)

from contextlib import ExitStack

import concourse.bass as bass
import concourse.bacc as bacc
import concourse.mybir as mybir
import concourse.tile as tile
from concourse.bass_utils import run_bass_kernel_spmd

FP32 = mybir.dt.float32
FP16 = mybir.dt.float16
ALU = mybir.AluOpType
ACTF = mybir.ActivationFunctionType
AXL = mybir.AxisListType

B, F, H = 16384, 64, 64
NCORES = 8
BL = B // NCORES            # 2048
NPAIR = F // 2              # 32
NCHUNK = 4
CK = BL // NCHUNK           # 512
NEG = 0.01
PRIOR_SCALE = 0.1
LOG_PS = float(np.log(PRIOR_SCALE))
INV_2S2 = 1.0 / (2.0 * PRIOR_SCALE ** 2)

PASS2_ACT_PAIRS = 6
WARMUP_MMS = 14

# (mu, exp(ls) tile key, ls, D)
KL_GROUPS = [
    ("wmu0", "e0", "wls0", 64.0),
    ("bmu0", "eb0t", "bls0", 64.0),
    ("wmu1", "e1", "wls1", 4096.0),
    ("bmu1", "eb1t", "bls1", 64.0),
    ("wmu2", "e2", "wls2", 64.0),
    ("bmu2", "eb2t", "bls2", 1.0),
]
KL_CONST = 6.0 * F * (LOG_PS - 0.5)

_BUILT = None
_last_in_maps = None


def _raw_ap(handle, offset, dims):
    return bass.AP(tensor=handle, offset=offset, ap=[list(d) for d in dims])


def build():
    nc = bacc.Bacc("TRN2", target_bir_lowering=False)

    f_in = nc.declare_dram_parameter("f", [BL, F], FP32, isOutput=False)
    P = {}
    for name, shape in [
        ("wmu0", [F, H]), ("wls0", [F, H]), ("ew0", [F, H]),
        ("bmu0", [F, H]), ("bls0", [F, H]), ("eb0", [F, H]),
        ("wmu1", [F, H, H]), ("wls1", [F, H, H]), ("ew1", [F, H, H]),
        ("bmu1", [F, H]), ("bls1", [F, H]), ("eb1", [F, H]),
        ("wmu2", [F, H]), ("wls2", [F, H]), ("ew2", [F, H]),
        ("bmu2", [F, 1]), ("bls2", [F, 1]),
    ]:
        P[name] = nc.declare_dram_parameter(name, shape, FP32, isOutput=False)
    y_out = nc.declare_dram_parameter("y_part", [NCHUNK, CK], FP32, isOutput=True)
    kl_out = nc.declare_dram_parameter("kl", [1, 1], FP32, isOutput=True)

    scr_t0 = nc.dram_tensor("scr_t0", [65 * 64 * H], FP16)
    scr_id = nc.dram_tensor("scr_id", [128 * 128], FP16)
    scr_co = nc.dram_tensor("scr_co", [3 * len(KL_GROUPS)], FP32)

    with tile.TileContext(nc) as tc:
        with ExitStack() as ctx:
            wpool = ctx.enter_context(tc.tile_pool(name="weights", bufs=1))
            a0p = ctx.enter_context(tc.tile_pool(name="a0", bufs=6))
            a1p = ctx.enter_context(tc.tile_pool(name="a1", bufs=3))
            ps0 = ctx.enter_context(tc.tile_pool(name="ps0", bufs=1, space="PSUM"))
            ps1 = ctx.enter_context(tc.tile_pool(name="ps1", bufs=2, space="PSUM"))
            psy = ctx.enter_context(tc.tile_pool(name="psy", bufs=1, space="PSUM"))
            psm = ctx.enter_context(tc.tile_pool(name="psmisc", bufs=1, space="PSUM"))
            spool = ctx.enter_context(tc.tile_pool(name="setup", bufs=1))

            # -------- early zeros + PE warmup (HAM) --------
            wz = spool.tile([128, CK], FP16, tag="wz")
            nc.vector.memset(wz[:], 0.0)
            warm_ps = psm.tile([128, CK], FP32, tag="psmisc")
            for _ in range(WARMUP_MMS):
                nc.tensor.matmul(warm_ps[:], wz[:, 0:128], wz[:], start=True, stop=True)

            # -------- input load (gpsimd casting DMA, issued first) --------
            x16all = spool.tile([128, 16 * 64], FP16, tag="x16all")
            nc.gpsimd.dma_start(
                x16all[:].rearrange("p (t f) -> p t f", t=16),
                _raw_ap(f_in, 0, [[F, 128], [128 * F, 16], [1, F]]))

            # -------- parameter loads --------
            # big layer-1 tensors: split across scalar-HWDGE and gpsimd queues
            g = {}
            big_engines = [nc.scalar, nc.gpsimd, nc.scalar, nc.gpsimd, nc.scalar, nc.gpsimd]
            bi = 0
            for name in ("wmu1", "wls1", "ew1"):
                t = spool.tile([128, NPAIR * H], FP32, tag=name)
                for par in (0, 1):
                    big_engines[bi].dma_start(
                        t[par * 64:(par + 1) * 64, :].rearrange("i (p o) -> i p o", p=NPAIR),
                        _raw_ap(P[name], par * H * H,
                                [[H, 64], [2 * H * H, NPAIR], [1, H]]))
                    bi += 1
                g[name] = t
            # small tensors on the sync queue (no upstream deps)
            for name in ("wmu0", "wls0", "ew0", "bmu0", "bls0", "eb0"):
                t = spool.tile([F, H], FP32, tag=name)
                nc.sync.dma_start(t[:], P[name][:])
                g[name] = t
            for name in ("bmu2", "bls2"):
                t = spool.tile([F, 1], FP32, tag=name)
                nc.sync.dma_start(t[:], P[name][:])
                g[name] = t
            for name in ("bmu1", "bls1", "eb1", "wmu2", "wls2", "ew2"):
                t = spool.tile([128, NPAIR], FP32, tag=name)
                for par in (0, 1):
                    nc.sync.dma_start(
                        t[par * 64:(par + 1) * 64, :],
                        _raw_ap(P[name], par * H, [[1, 64], [2 * H, NPAIR]]))
                g[name] = t

            # -------- layer-0 effective weights + T0all bounce --------
            e0 = spool.tile([F, H], FP32, tag="e0")
            nc.scalar.activation(e0[:], g["wls0"][:], ACTF.Exp)
            w0e = spool.tile([F, H], FP32, tag="w0e")
            nc.vector.tensor_mul(w0e[:], e0[:], g["ew0"][:])
            nc.vector.tensor_add(w0e[:], w0e[:], g["wmu0"][:])
            w0e16 = spool.tile([F, H], FP16, tag="w0e16")
            nc.vector.tensor_copy(w0e16[:], w0e[:])

            eb0t = spool.tile([F, H], FP32, tag="eb0t")
            nc.scalar.activation(eb0t[:], g["bls0"][:], ACTF.Exp)
            b0e = spool.tile([F, H], FP32, tag="b0e")
            nc.vector.tensor_mul(b0e[:], eb0t[:], g["eb0"][:])
            nc.vector.tensor_add(b0e[:], b0e[:], g["bmu0"][:])
            b0e16 = spool.tile([F, H], FP16, tag="b0e16")
            nc.vector.tensor_copy(b0e16[:], b0e[:])

            # bounce: zero scratch (8 strips from the zero tile), diag W0e,
            # bias row, identity diag (sync queue)
            for s in range(8):
                nc.sync.dma_start(
                    _raw_ap(scr_t0, s * CK, [[64 * H, 65], [1, CK]]), wz[0:65, :])
            nc.sync.dma_start(_raw_ap(scr_id, 0, [[128, 128], [1, 128]]),
                              wz[0:128, 0:128])
            nc.sync.dma_start(_raw_ap(scr_t0, 0, [[64 * H + H, F], [1, H]]), w0e16[:])
            nc.sync.dma_start(_raw_ap(scr_t0, 64 * 64 * H, [[H, F], [1, H]]), b0e16[:])
            ones16r = spool.tile([1, 128], FP16, tag="ones16r")
            nc.vector.memset(ones16r[:], 1.0)
            nc.sync.dma_start(_raw_ap(scr_id, 0, [[129, 128]]), ones16r[:])

            t0all = wpool.tile([65, 64 * H], FP16, tag="t0all")
            nc.sync.dma_start(t0all[:], _raw_ap(scr_t0, 0, [[64 * H, 65], [1, 64 * H]]))
            ident = wpool.tile([128, 128], FP16, tag="ident")
            nc.sync.dma_start(ident[:], _raw_ap(scr_id, 0, [[128, 128], [1, 128]]))

            # -------- X_T via PE transposes (warmup sprinkled between) --------
            xT = wpool.tile([65, BL], FP16, tag="xT")
            nc.vector.memset(xT[64:65, :], 1.0)
            for t in range(16):
                pst = ps1.tile([64, 128], FP16, tag="p1")
                nc.tensor.transpose(pst[:], x16all[:, t * 64:(t + 1) * 64], ident[:])
                nc.vector.tensor_copy(xT[0:64, t * 128:(t + 1) * 128], pst[:])
                if t % 4 == 3:
                    nc.tensor.matmul(warm_ps[:], wz[:, 0:128], wz[:],
                                     start=True, stop=True)

            # -------- layer-1/2 effective weights --------
            e1 = spool.tile([128, NPAIR * H], FP32, tag="e1")
            nc.scalar.activation(e1[:], g["wls1"][:], ACTF.Exp)
            w1e = spool.tile([128, NPAIR * H], FP32, tag="w1e")
            nc.vector.tensor_mul(w1e[:], e1[:], g["ew1"][:])
            nc.vector.tensor_add(w1e[:], w1e[:], g["wmu1"][:])

            w1blk = wpool.tile([128, NPAIR * 128], FP16, tag="w1blk")
            nc.vector.memset(w1blk[:], 0.0)
            nc.gpsimd.dma_start(
                w1blk[0:64, :].rearrange("i (p o) -> i p o", o=128)[:, :, 0:64],
                w1e[0:64, :].rearrange("i (p o) -> i p o", o=64))
            nc.gpsimd.dma_start(
                w1blk[64:128, :].rearrange("i (p o) -> i p o", o=128)[:, :, 64:128],
                w1e[64:128, :].rearrange("i (p o) -> i p o", o=64))

            eb1t = spool.tile([128, NPAIR], FP32, tag="eb1t")
            nc.scalar.activation(eb1t[:], g["bls1"][:], ACTF.Exp)
            b1e = wpool.tile([128, NPAIR], FP32, tag="b1e")
            nc.vector.tensor_mul(b1e[:], eb1t[:], g["eb1"][:])
            nc.vector.tensor_add(b1e[:], b1e[:], g["bmu1"][:])

            e2 = spool.tile([128, NPAIR], FP32, tag="e2")
            nc.scalar.activation(e2[:], g["wls2"][:], ACTF.Exp)
            w2e = spool.tile([128, NPAIR], FP32, tag="w2e")
            nc.vector.tensor_mul(w2e[:], e2[:], g["ew2"][:])
            nc.vector.tensor_add(w2e[:], w2e[:], g["wmu2"][:])
            w2sb = wpool.tile([128, NPAIR], FP16, tag="w2sb")
            nc.vector.tensor_copy(w2sb[:], w2e[:])
            w2sb99 = wpool.tile([128, NPAIR], FP16, tag="w2sb99")
            nc.vector.tensor_scalar(w2sb99[:], w2e[:], 1.0 - NEG, None, ALU.mult)
            w2sb01 = wpool.tile([128, NPAIR], FP16, tag="w2sb01")
            nc.vector.tensor_scalar(w2sb01[:], w2e[:], NEG, None, ALU.mult)
            eb2t = spool.tile([F, 1], FP32, tag="eb2t")
            nc.scalar.activation(eb2t[:], g["bls2"][:], ACTF.Exp)

            # -------- KL sums on DVE (reuse exp tiles) --------
            acc = spool.tile([128, 3 * len(KL_GROUPS)], FP32, tag="acc")
            nc.vector.memset(acc[:], 0.0)
            dump_big = spool.tile([128, NPAIR * H], FP32, tag="dump_big")
            dump_small = spool.tile([128, H], FP32, tag="dump_small")
            for gi, (mun, en, lsn, _D) in enumerate(KL_GROUPS):
                mu, ex, ls = g[mun], None, g[lsn]
                ex = {"e0": e0, "eb0t": eb0t, "e1": e1, "eb1t": eb1t,
                      "e2": e2, "eb2t": eb2t}[en]
                parts = mu.shape[0]
                big = mu.shape[1] > H
                dmp = dump_big if big else dump_small
                d = dmp[0:parts, 0:mu.shape[1]]
                nc.vector.scalar_tensor_tensor(
                    d, mu[:], 1.0, mu[:], ALU.mult, ALU.mult,
                    accum_out=acc[0:parts, 3 * gi:3 * gi + 1])
                nc.vector.scalar_tensor_tensor(
                    d, ex[:], 1.0, ex[:], ALU.mult, ALU.mult,
                    accum_out=acc[0:parts, 3 * gi + 1:3 * gi + 2])
                nc.vector.tensor_reduce(
                    acc[0:parts, 3 * gi + 2:3 * gi + 3], ls[:], AXL.X, ALU.add)

            # -------- main loop --------
            psum_y = psy.tile([128, CK], FP32, tag="psum_y")
            w1T = wpool.tile([128, NPAIR * 128], FP16, tag="w1T")
            v1ps = psm.tile([128, NPAIR], FP32, tag="psmisc")
            v1sb = wpool.tile([128, NPAIR], FP16, tag="v1sb")

            def emit_l2(p, a1, a0s, last):
                act_p = p < PASS2_ACT_PAIRS
                w2x = w2sb if act_p else w2sb99
                for c in range(NCHUNK):
                    nc.tensor.matmul(psum_y[32 * c:32 * c + 1, :],
                                     w2x[:, p:p + 1],
                                     a1[:, c * CK:(c + 1) * CK],
                                     start=(p == 0), stop=(last and act_p),
                                     tile_position=(0, 32 * c),
                                     skip_group_check=True)
                if not act_p:
                    for c in range(NCHUNK):
                        h, j = divmod(c, 2)
                        nc.tensor.matmul(psum_y[32 * c:32 * c + 1, :],
                                         v1sb[:, p:p + 1],
                                         a0s[h][:, j * CK:(j + 1) * CK],
                                         start=False, stop=last,
                                         tile_position=(0, 32 * c),
                                         skip_group_check=True)

            prev = None
            for p in range(NPAIR):
                act_pass2 = p < PASS2_ACT_PAIRS
                a1 = a1p.tile([128, BL], FP16, tag="a1")
                a0s = []
                for h in range(2):
                    p0 = ps0.tile([128, 1024], FP32, tag="p0")
                    for j in range(2):
                        c = 2 * h + j
                        nc.tensor.matmul(
                            p0[:, j * CK:(j + 1) * CK],
                            t0all[0:65, 128 * p:128 * (p + 1)],
                            xT[0:65, c * CK:(c + 1) * CK],
                            start=True, stop=True)
                    a0 = a0p.tile([128, 1024], FP16, tag="a0")
                    a0s.append(a0)
                    nc.scalar.activation(a0[:], p0[:], ACTF.Lrelu,
                                         scale=1.0, alpha=NEG)
                    p1 = ps1.tile([128, 1024], FP32, tag="p1")
                    for j in range(2):
                        nc.tensor.matmul(p1[:, j * CK:(j + 1) * CK],
                                         w1blk[:, 128 * p:128 * (p + 1)],
                                         a0[:, j * CK:(j + 1) * CK],
                                         start=True, stop=True)
                    if act_pass2:
                        nc.scalar.activation(a1[:, 1024 * h:1024 * (h + 1)], p1[:],
                                             ACTF.Lrelu, bias=b1e[:, p:p + 1],
                                             scale=1.0, alpha=NEG)
                    else:
                        nc.vector.tensor_scalar(a1[:, 1024 * h:1024 * (h + 1)], p1[:],
                                                b1e[:, p:p + 1], 0.0,
                                                ALU.add, ALU.max)
                if prev is not None:
                    emit_l2(*prev, last=False)
                prev = (p, a1, a0s)
                # interleave W1T transposes + V1 matmuls into early pairs
                if 1 <= p <= 4:
                    for q in range(8 * (p - 1), 8 * p):
                        pst = ps1.tile([128, 128], FP16, tag="p1")
                        nc.tensor.transpose(pst[:], w1blk[:, 128 * q:128 * (q + 1)],
                                            ident[:])
                        nc.vector.tensor_copy(w1T[:, 128 * q:128 * (q + 1)], pst[:])
                        nc.tensor.matmul(v1ps[:, q:q + 1],
                                         w1T[:, 128 * q:128 * (q + 1)],
                                         w2sb01[:, q:q + 1], start=True, stop=True,
                                         skip_group_check=True)
                if p == 5:
                    nc.vector.tensor_copy(v1sb[:], v1ps[:])
            emit_l2(*prev, last=True)

            # -------- y evacuation --------
            ysb = spool.tile([128, CK], FP32, tag="ysb")
            nc.vector.tensor_copy(ysb[:], psum_y[:])
            for c in range(NCHUNK):
                nc.sync.dma_start(y_out[c:c + 1, :], ysb[32 * c:32 * c + 1, :])

            # -------- KL assembly (after loop) --------
            ones32 = spool.tile([128, 1], FP32, tag="ones32")
            nc.vector.memset(ones32[:], 1.0)
            nacc = 3 * len(KL_GROUPS)
            kl_ps = psm.tile([nacc, 1], FP32, tag="psmisc")
            nc.tensor.matmul(kl_ps[:], acc[:], ones32[:], start=True, stop=True)
            kl_cols = spool.tile([nacc, 1], FP32, tag="kl_cols")
            nc.vector.tensor_copy(kl_cols[:], kl_ps[:])

            corow = spool.tile([1, nacc], FP32, tag="corow")
            for gi, (_m, _e, _l, D) in enumerate(KL_GROUPS):
                nc.vector.memset(corow[0:1, 3 * gi:3 * gi + 1], INV_2S2 / D)
                nc.vector.memset(corow[0:1, 3 * gi + 1:3 * gi + 2], INV_2S2 / D)
                nc.vector.memset(corow[0:1, 3 * gi + 2:3 * gi + 3], -1.0 / D)
            nc.sync.dma_start(_raw_ap(scr_co, 0, [[1, nacc]]), corow[:])
            cocol = spool.tile([nacc, 1], FP32, tag="cocol")
            nc.sync.dma_start(cocol[:], _raw_ap(scr_co, 0, [[1, nacc], [1, 1]]))

            kl_ps2 = psm.tile([1, 1], FP32, tag="psmisc")
            nc.tensor.matmul(kl_ps2[:], kl_cols[:], cocol[:], start=True, stop=True)
            kl_sb = spool.tile([1, 1], FP32, tag="kl_sb")
            nc.vector.tensor_scalar(kl_sb[:], kl_ps2[:], KL_CONST, None, ALU.add)
            nc.sync.dma_start(kl_out[:], kl_sb[:])

    if not nc.is_finalized():
        nc.finalize()
    return nc


def _get_built():
    global _BUILT
    if _BUILT is None:
        _BUILT = build()
    return _BUILT


def kernel(**inputs):
    inputs = {k: np.ascontiguousarray(np.asarray(v), dtype=np.float32)
              for k, v in inputs.items()}
    nc = _get_built()

    f = inputs["f"]
    pnames = ["wmu0", "wls0", "ew0", "bmu0", "bls0", "eb0",
              "wmu1", "wls1", "ew1", "bmu1", "bls1", "eb1",
              "wmu2", "wls2", "ew2", "bmu2", "bls2"]
    base = {}
    for n in pnames:
        v = inputs[n]
        if n in ("wmu0", "wls0", "ew0", "wmu2", "wls2", "ew2"):
            v = v.reshape(F, H)
        elif n in ("bmu2", "bls2"):
            v = v.reshape(F, 1)
        base[n] = np.ascontiguousarray(v)

    in_maps = []
    for c in range(NCORES):
        m = dict(base)
        m["f"] = np.ascontiguousarray(f[c * BL:(c + 1) * BL, :])
        in_maps.append(m)
    global _last_in_maps
    _last_in_maps = in_maps

    res = run_bass_kernel_spmd(nc, in_maps, list(range(NCORES)))

    b2e = inputs["bmu2"][:, 0] + np.exp(inputs["bls2"][:, 0]) * inputs["eb2"][:, 0]
    w2e = (inputs["wmu2"] + np.exp(inputs["wls2"]) * inputs["ew2"]).reshape(F, H)
    b1e = inputs["bmu1"] + np.exp(inputs["bls1"]) * inputs["eb1"]
    dve_feats = np.arange(2 * PASS2_ACT_PAIRS, F)
    lin_const = NEG * np.float32(
        (w2e[dve_feats].astype(np.float16).astype(np.float32)
         * b1e[dve_feats].astype(np.float32)).sum())
    yconst = np.float32(b2e.sum() + inputs["bias"][0] + lin_const)
    ys = []
    for c in range(NCORES):
        part = np.asarray(res.results[c]["y_part"])
        ys.append(part.reshape(BL) + yconst)
    y = np.concatenate(ys).astype(np.float32)
    kl = np.float32(np.asarray(res.results[0]["kl"]).reshape(()))
    return y, kl


# revision 12
# speedup vs baseline: 1.3685x; 1.3685x over previous
"""BayesNAM forward (dense per-feature MLP ensemble) on 8 Trainium2 NeuronCores.

Math (per batch row b, feature f; H=64 hidden):
    A0 = leaky(x[b,f] * W0e[f,:] + b0e[f,:])          W0e = wmu0 + exp(wls0)*ew0 ...
    A1 = leaky(A0 @ W1e[f] + b1e[f])
    y[b] = sum_f (A1 @ W2e[f] + b2e[f]) + bias
    kl   = scalar function of (mu, log_sigma) params only

Sharding: batch (16384) split across 8 cores (2048 each); parameters replicated.

Per-core kernel (fp16 matmul inputs, fp32 accumulation):
  X_T   [65, 2048] fp16  : transposed input; row 64 = ones (layer-0 bias row)
  T0all [65, 4096] fp16  : layer-0 weights; feature f's row diag-placed at cols
                           [64f, 64f+64) so one K=65 matmul computes a feature
                           PAIR's layer-0 (+bias); built via a DRAM bounce
  W1BLK [128, 4096] fp16 : layer-1 weights as block-diagonal [128,128] tiles
  W1T   [128, 4096] fp16 : PE-transposed W1BLK blocks, for V1 = 0.01*W1@w2
  pair loop (layer-2 emitted one pair behind to keep the in-order PE fed):
    l0 matmul (K=65, bias via ones row) -> pass1 ACT Lrelu [128,1024] ->
    l1 blockdiag matmul -> pass2: ACT Lrelu(+bias) for a few pairs, else
    DVE relu(z+b1) with the 0.01-linear leak restored by V1^T@A0 matmuls
    and a host constant -> M=1 matmuls accumulate y into one PSUM bank,
    column-packed 4-way by chunk.
  KL sums ride DVE (squares of the already-computed exp tiles); reduction
  matmuls run after the loop. y partial rows and constants combined on host.
"""
import sys
import numpy as np

for _p in ("/opt/trn_rl_repo", "/root/.axon_site/_ro/trn_rl_repo"):
    if _p not in sys.path:
        sys.path.append(_p)

from contextlib import ExitStack

import concourse.bass as bass
import concourse.bacc as bacc
import concourse.mybir as mybir
import concourse.tile as tile
from concourse.bass_utils import run_bass_kernel_spmd

FP32 = mybir.dt.float32
FP16 = mybir.dt.float16
ALU = mybir.AluOpType
ACTF = mybir.ActivationFunctionType
AXL = mybir.AxisListType

B, F, H = 16384, 64, 64
NCORES = 8
BL = B // NCORES            # 2048
NPAIR = F // 2              # 32
NCHUNK = 4
CK = BL // NCHUNK           # 512
NEG = 0.01
PRIOR_SCALE = 0.1
LOG_PS = float(np.log(PRIOR_SCALE))
INV_2S2 = 1.0 / (2.0 * PRIOR_SCALE ** 2)

PASS2_ACT_PAIRS = 6
WARMUP_MMS = 14

# (mu, exp(ls) tile key, ls, D)
KL_GROUPS = [
    ("wmu0", "e0", "wls0", 64.0),
    ("bmu0", "eb0t", "bls0", 64.0),
    ("wmu1", "e1", "wls1", 4096.0),
    ("bmu1", "eb1t", "bls1", 64.0),
    ("wmu2", "e2", "wls2", 64.0),
    ("bmu2", "eb2t", "bls2", 1.0),
]
KL_CONST = 6.0 * F * (LOG_PS - 0.5)

_BUILT = None
_last_in_maps = None


def _raw_ap(handle, offset, dims):
    return bass.AP(tensor=handle, offset=offset, ap=[list(d) for d in dims])


def build():
    nc = bacc.Bacc("TRN2", target_bir_lowering=False)

    f_in = nc.declare_dram_parameter("f", [BL, F], FP32, isOutput=False)
    P = {}
    for name, shape in [
        ("wmu0", [F, H]), ("wls0", [F, H]), ("ew0", [F, H]),
        ("bmu0", [F, H]), ("bls0", [F, H]), ("eb0", [F, H]),
        ("wmu1", [F, H, H]), ("wls1", [F, H, H]), ("ew1", [F, H, H]),
        ("bmu1", [F, H]), ("bls1", [F, H]), ("eb1", [F, H]),
        ("wmu2", [F, H]), ("wls2", [F, H]), ("ew2", [F, H]),
        ("bmu2", [F, 1]), ("bls2", [F, 1]),
    ]:
        P[name] = nc.declare_dram_parameter(name, shape, FP32, isOutput=False)
    y_out = nc.declare_dram_parameter("y_part", [NCHUNK, CK], FP32, isOutput=True)
    kl_out = nc.declare_dram_parameter("kl", [1, 1], FP32, isOutput=True)

    scr_t0 = nc.dram_tensor("scr_t0", [65 * 64 * H], FP16)
    scr_id = nc.dram_tensor("scr_id", [128 * 128], FP16)
    scr_co = nc.dram_tensor("scr_co", [3 * len(KL_GROUPS)], FP32)

    with tile.TileContext(nc) as tc:
        with ExitStack() as ctx:
            wpool = ctx.enter_context(tc.tile_pool(name="weights", bufs=1))
            a0p = ctx.enter_context(tc.tile_pool(name="a0", bufs=6))
            a1p = ctx.enter_context(tc.tile_pool(name="a1", bufs=3))
            ps0 = ctx.enter_context(tc.tile_pool(name="ps0", bufs=2, space="PSUM"))
            ps1 = ctx.enter_context(tc.tile_pool(name="ps1", bufs=2, space="PSUM"))
            psy = ctx.enter_context(tc.tile_pool(name="psy", bufs=1, space="PSUM"))
            psm = ctx.enter_context(tc.tile_pool(name="psmisc", bufs=1, space="PSUM"))
            spool = ctx.enter_context(tc.tile_pool(name="setup", bufs=1))

            # -------- early zeros + PE warmup (HAM) --------
            wz = spool.tile([128, CK], FP16, tag="wz")
            nc.vector.memset(wz[:], 0.0)
            warm_ps = psm.tile([128, CK], FP32, tag="psmisc")
            for _ in range(WARMUP_MMS):
                nc.tensor.matmul(warm_ps[:], wz[:, 0:128], wz[:], start=True, stop=True)

            # -------- input load (gpsimd casting DMA, issued first) --------
            x16all = spool.tile([128, 16 * 64], FP16, tag="x16all")
            nc.gpsimd.dma_start(
                x16all[:].rearrange("p (t f) -> p t f", t=16),
                _raw_ap(f_in, 0, [[F, 128], [128 * F, 16], [1, F]]))

            # -------- parameter loads --------
            # big layer-1 tensors: split across scalar-HWDGE and gpsimd queues
            g = {}
            big_engines = [nc.scalar, nc.gpsimd, nc.scalar, nc.gpsimd, nc.scalar, nc.gpsimd]
            bi = 0
            for name in ("wmu1", "wls1", "ew1"):
                t = spool.tile([128, NPAIR * H], FP32, tag=name)
                for par in (0, 1):
                    big_engines[bi].dma_start(
                        t[par * 64:(par + 1) * 64, :].rearrange("i (p o) -> i p o", p=NPAIR),
                        _raw_ap(P[name], par * H * H,
                                [[H, 64], [2 * H * H, NPAIR], [1, H]]))
                    bi += 1
                g[name] = t
            # small tensors spread over sync/scalar queues (no upstream deps)
            _di = [0]
            def sdma(out_ap, in_ap):
                eng = (nc.sync, nc.scalar)[_di[0] % 2]
                _di[0] += 1
                eng.dma_start(out_ap, in_ap)
            for name in ("wmu0", "wls0", "ew0", "bmu0", "bls0", "eb0"):
                t = spool.tile([F, H], FP32, tag=name)
                sdma(t[:], P[name][:])
                g[name] = t
            for name in ("bmu2", "bls2"):
                t = spool.tile([F, 1], FP32, tag=name)
                sdma(t[:], P[name][:])
                g[name] = t
            for name in ("bmu1", "bls1", "eb1", "wmu2", "wls2", "ew2"):
                t = spool.tile([128, NPAIR], FP32, tag=name)
                for par in (0, 1):
                    sdma(t[par * 64:(par + 1) * 64, :],
                         _raw_ap(P[name], par * H, [[1, 64], [2 * H, NPAIR]]))
                g[name] = t

            # -------- layer-0 effective weights + T0all bounce --------
            e0 = spool.tile([F, H], FP32, tag="e0")
            nc.scalar.activation(e0[:], g["wls0"][:], ACTF.Exp)
            w0e = spool.tile([F, H], FP32, tag="w0e")
            nc.vector.tensor_mul(w0e[:], e0[:], g["ew0"][:])
            nc.vector.tensor_add(w0e[:], w0e[:], g["wmu0"][:])
            w0e16 = spool.tile([F, H], FP16, tag="w0e16")
            nc.vector.tensor_copy(w0e16[:], w0e[:])

            eb0t = spool.tile([F, H], FP32, tag="eb0t")
            nc.scalar.activation(eb0t[:], g["bls0"][:], ACTF.Exp)
            b0e = spool.tile([F, H], FP32, tag="b0e")
            nc.vector.tensor_mul(b0e[:], eb0t[:], g["eb0"][:])
            nc.vector.tensor_add(b0e[:], b0e[:], g["bmu0"][:])
            b0e16 = spool.tile([F, H], FP16, tag="b0e16")
            nc.vector.tensor_copy(b0e16[:], b0e[:])

            # bounce: zero scratch (8 strips from the zero tile), diag W0e,
            # bias row, identity diag (sync queue)
            for s in range(8):
                eng = (nc.sync, nc.scalar)[s % 2]
                eng.dma_start(
                    _raw_ap(scr_t0, s * CK, [[64 * H, 65], [1, CK]]), wz[0:65, :])
            nc.scalar.dma_start(_raw_ap(scr_id, 0, [[128, 128], [1, 128]]),
                                wz[0:128, 0:128])
            nc.sync.dma_start(_raw_ap(scr_t0, 0, [[64 * H + H, F], [1, H]]), w0e16[:])
            nc.sync.dma_start(_raw_ap(scr_t0, 64 * 64 * H, [[H, F], [1, H]]), b0e16[:])
            ones16r = spool.tile([1, 128], FP16, tag="ones16r")
            nc.vector.memset(ones16r[:], 1.0)
            nc.sync.dma_start(_raw_ap(scr_id, 0, [[129, 128]]), ones16r[:])

            t0all = wpool.tile([65, 64 * H], FP16, tag="t0all")
            nc.sync.dma_start(t0all[:], _raw_ap(scr_t0, 0, [[64 * H, 65], [1, 64 * H]]))
            ident = wpool.tile([128, 128], FP16, tag="ident")
            nc.sync.dma_start(ident[:], _raw_ap(scr_id, 0, [[128, 128], [1, 128]]))

            # -------- X_T via PE transposes (warmup sprinkled between) --------
            xT = wpool.tile([65, BL], FP16, tag="xT")
            nc.vector.memset(xT[64:65, :], 1.0)
            for t in range(16):
                pst = ps1.tile([64, 128], FP16, tag="p1")
                nc.tensor.transpose(pst[:], x16all[:, t * 64:(t + 1) * 64], ident[:])
                nc.vector.tensor_copy(xT[0:64, t * 128:(t + 1) * 128], pst[:])
                if t % 4 == 3:
                    nc.tensor.matmul(warm_ps[:], wz[:, 0:128], wz[:],
                                     start=True, stop=True)

            # -------- layer-1/2 effective weights --------
            e1 = spool.tile([128, NPAIR * H], FP32, tag="e1")
            nc.scalar.activation(e1[:], g["wls1"][:], ACTF.Exp)
            w1e = spool.tile([128, NPAIR * H], FP32, tag="w1e")
            nc.vector.tensor_mul(w1e[:], e1[:], g["ew1"][:])
            nc.vector.tensor_add(w1e[:], w1e[:], g["wmu1"][:])

            w1blk = wpool.tile([128, NPAIR * 128], FP16, tag="w1blk")
            nc.vector.memset(w1blk[:], 0.0)
            nc.gpsimd.dma_start(
                w1blk[0:64, :].rearrange("i (p o) -> i p o", o=128)[:, :, 0:64],
                w1e[0:64, :].rearrange("i (p o) -> i p o", o=64))
            nc.gpsimd.dma_start(
                w1blk[64:128, :].rearrange("i (p o) -> i p o", o=128)[:, :, 64:128],
                w1e[64:128, :].rearrange("i (p o) -> i p o", o=64))

            eb1t = spool.tile([128, NPAIR], FP32, tag="eb1t")
            nc.scalar.activation(eb1t[:], g["bls1"][:], ACTF.Exp)
            b1e = wpool.tile([128, NPAIR], FP32, tag="b1e")
            nc.vector.tensor_mul(b1e[:], eb1t[:], g["eb1"][:])
            nc.vector.tensor_add(b1e[:], b1e[:], g["bmu1"][:])

            e2 = spool.tile([128, NPAIR], FP32, tag="e2")
            nc.scalar.activation(e2[:], g["wls2"][:], ACTF.Exp)
            w2e = spool.tile([128, NPAIR], FP32, tag="w2e")
            nc.vector.tensor_mul(w2e[:], e2[:], g["ew2"][:])
            nc.vector.tensor_add(w2e[:], w2e[:], g["wmu2"][:])
            w2sb = wpool.tile([128, NPAIR], FP16, tag="w2sb")
            nc.vector.tensor_copy(w2sb[:], w2e[:])
            w2sb99 = wpool.tile([128, NPAIR], FP16, tag="w2sb99")
            nc.vector.tensor_scalar(w2sb99[:], w2e[:], 1.0 - NEG, None, ALU.mult)
            w2sb01 = wpool.tile([128, NPAIR], FP16, tag="w2sb01")
            nc.vector.tensor_scalar(w2sb01[:], w2e[:], NEG, None, ALU.mult)
            eb2t = spool.tile([F, 1], FP32, tag="eb2t")
            nc.scalar.activation(eb2t[:], g["bls2"][:], ACTF.Exp)

            # -------- KL sums on DVE (reuse exp tiles) --------
            acc = spool.tile([128, 3 * len(KL_GROUPS)], FP32, tag="acc")
            nc.vector.memset(acc[:], 0.0)
            dump_big = spool.tile([128, NPAIR * H], FP32, tag="dump_big")
            dump_small = spool.tile([128, H], FP32, tag="dump_small")
            for gi, (mun, en, lsn, _D) in enumerate(KL_GROUPS):
                mu, ex, ls = g[mun], None, g[lsn]
                ex = {"e0": e0, "eb0t": eb0t, "e1": e1, "eb1t": eb1t,
                      "e2": e2, "eb2t": eb2t}[en]
                parts = mu.shape[0]
                big = mu.shape[1] > H
                dmp = dump_big if big else dump_small
                d = dmp[0:parts, 0:mu.shape[1]]
                nc.vector.scalar_tensor_tensor(
                    d, mu[:], 1.0, mu[:], ALU.mult, ALU.mult,
                    accum_out=acc[0:parts, 3 * gi:3 * gi + 1])
                nc.vector.scalar_tensor_tensor(
                    d, ex[:], 1.0, ex[:], ALU.mult, ALU.mult,
                    accum_out=acc[0:parts, 3 * gi + 1:3 * gi + 2])
                nc.vector.tensor_reduce(
                    acc[0:parts, 3 * gi + 2:3 * gi + 3], ls[:], AXL.X, ALU.add)

            # -------- main loop --------
            psum_y = psy.tile([128, CK], FP32, tag="psum_y")
            w1T = wpool.tile([128, NPAIR * 128], FP16, tag="w1T")
            v1ps = psm.tile([128, NPAIR], FP32, tag="psmisc")
            v1sb = wpool.tile([128, NPAIR], FP16, tag="v1sb")

            def emit_l2(p, a1, a0s, last):
                act_p = p < PASS2_ACT_PAIRS
                w2x = w2sb if act_p else w2sb99
                for c in range(NCHUNK):
                    nc.tensor.matmul(psum_y[32 * c:32 * c + 1, :],
                                     w2x[:, p:p + 1],
                                     a1[:, c * CK:(c + 1) * CK],
                                     start=(p == 0), stop=(last and act_p),
                                     tile_position=(0, 32 * c),
                                     skip_group_check=True)
                if not act_p:
                    for c in range(NCHUNK):
                        h, j = divmod(c, 2)
                        nc.tensor.matmul(psum_y[32 * c:32 * c + 1, :],
                                         v1sb[:, p:p + 1],
                                         a0s[h][:, j * CK:(j + 1) * CK],
                                         start=False, stop=last,
                                         tile_position=(0, 32 * c),
                                         skip_group_check=True)

            prev = None
            for p in range(NPAIR):
                act_pass2 = p < PASS2_ACT_PAIRS
                a1 = a1p.tile([128, BL], FP16, tag="a1")
                a0s = []
                for h in range(2):
                    p0 = ps0.tile([128, 1024], FP32, tag="p0")
                    for j in range(2):
                        c = 2 * h + j
                        nc.tensor.matmul(
                            p0[:, j * CK:(j + 1) * CK],
                            t0all[0:65, 128 * p:128 * (p + 1)],
                            xT[0:65, c * CK:(c + 1) * CK],
                            start=True, stop=True)
                    a0 = a0p.tile([128, 1024], FP16, tag="a0")
                    a0s.append(a0)
                    nc.scalar.activation(a0[:], p0[:], ACTF.Lrelu,
                                         scale=1.0, alpha=NEG)
                    for j in range(2):
                        c = 2 * h + j
                        p1 = ps1.tile([128, CK], FP32, tag="p1")
                        nc.tensor.matmul(p1[:],
                                         w1blk[:, 128 * p:128 * (p + 1)],
                                         a0[:, j * CK:(j + 1) * CK],
                                         start=True, stop=True)
                        if act_pass2:
                            nc.scalar.activation(a1[:, c * CK:(c + 1) * CK], p1[:],
                                                 ACTF.Lrelu, bias=b1e[:, p:p + 1],
                                                 scale=1.0, alpha=NEG)
                        else:
                            nc.vector.tensor_scalar(a1[:, c * CK:(c + 1) * CK], p1[:],
                                                    b1e[:, p:p + 1], 0.0,
                                                    ALU.add, ALU.max)
                if prev is not None:
                    emit_l2(*prev, last=False)
                prev = (p, a1, a0s)
                # interleave W1T transposes + V1 matmuls into early pairs
                if 1 <= p <= 4:
                    qs = [q for q in range(NPAIR) if q >= PASS2_ACT_PAIRS]
                    lo = (len(qs) * (p - 1)) // 4
                    hi = (len(qs) * p) // 4
                    for q in qs[lo:hi]:
                        pst = ps1.tile([128, 128], FP16, tag="p1")
                        nc.tensor.transpose(pst[:], w1blk[:, 128 * q:128 * (q + 1)],
                                            ident[:])
                        nc.vector.tensor_copy(w1T[:, 128 * q:128 * (q + 1)], pst[:])
                        nc.tensor.matmul(v1ps[:, q:q + 1],
                                         w1T[:, 128 * q:128 * (q + 1)],
                                         w2sb01[:, q:q + 1], start=True, stop=True,
                                         skip_group_check=True)
                if p == 5:
                    nc.vector.tensor_copy(v1sb[:], v1ps[:])
            emit_l2(*prev, last=True)

            # -------- y evacuation --------
            ysb = spool.tile([128, CK], FP32, tag="ysb")
            nc.vector.tensor_copy(ysb[:], psum_y[:])
            for c in range(NCHUNK):
                nc.sync.dma_start(y_out[c:c + 1, :], ysb[32 * c:32 * c + 1, :])

            # -------- KL assembly (after loop) --------
            ones32 = spool.tile([128, 1], FP32, tag="ones32")
            nc.vector.memset(ones32[:], 1.0)
            nacc = 3 * len(KL_GROUPS)
            kl_ps = psm.tile([nacc, 1], FP32, tag="psmisc")
            nc.tensor.matmul(kl_ps[:], acc[:], ones32[:], start=True, stop=True)
            kl_cols = spool.tile([nacc, 1], FP32, tag="kl_cols")
            nc.vector.tensor_copy(kl_cols[:], kl_ps[:])

            corow = spool.tile([1, nacc], FP32, tag="corow")
            for gi, (_m, _e, _l, D) in enumerate(KL_GROUPS):
                nc.vector.memset(corow[0:1, 3 * gi:3 * gi + 1], INV_2S2 / D)
                nc.vector.memset(corow[0:1, 3 * gi + 1:3 * gi + 2], INV_2S2 / D)
                nc.vector.memset(corow[0:1, 3 * gi + 2:3 * gi + 3], -1.0 / D)
            nc.sync.dma_start(_raw_ap(scr_co, 0, [[1, nacc]]), corow[:])
            cocol = spool.tile([nacc, 1], FP32, tag="cocol")
            nc.sync.dma_start(cocol[:], _raw_ap(scr_co, 0, [[1, nacc], [1, 1]]))

            kl_ps2 = psm.tile([1, 1], FP32, tag="psmisc")
            nc.tensor.matmul(kl_ps2[:], kl_cols[:], cocol[:], start=True, stop=True)
            kl_sb = spool.tile([1, 1], FP32, tag="kl_sb")
            nc.vector.tensor_scalar(kl_sb[:], kl_ps2[:], KL_CONST, None, ALU.add)
            nc.sync.dma_start(kl_out[:], kl_sb[:])

    if not nc.is_finalized():
        nc.finalize()
    return nc


def _get_built():
    global _BUILT
    if _BUILT is None:
        _BUILT = build()
    return _BUILT


def kernel(**inputs):
    inputs = {k: np.ascontiguousarray(np.asarray(v), dtype=np.float32)
              for k, v in inputs.items()}
    nc = _get_built()

    f = inputs["f"]
    pnames = ["wmu0", "wls0", "ew0", "bmu0", "bls0", "eb0",
              "wmu1", "wls1", "ew1", "bmu1", "bls1", "eb1",
              "wmu2", "wls2", "ew2", "bmu2", "bls2"]
    base = {}
    for n in pnames:
        v = inputs[n]
        if n in ("wmu0", "wls0", "ew0", "wmu2", "wls2", "ew2"):
            v = v.reshape(F, H)
        elif n in ("bmu2", "bls2"):
            v = v.reshape(F, 1)
        base[n] = np.ascontiguousarray(v)

    in_maps = []
    for c in range(NCORES):
        m = dict(base)
        m["f"] = np.ascontiguousarray(f[c * BL:(c + 1) * BL, :])
        in_maps.append(m)
    global _last_in_maps
    _last_in_maps = in_maps

    res = run_bass_kernel_spmd(nc, in_maps, list(range(NCORES)))

    b2e = inputs["bmu2"][:, 0] + np.exp(inputs["bls2"][:, 0]) * inputs["eb2"][:, 0]
    w2e = (inputs["wmu2"] + np.exp(inputs["wls2"]) * inputs["ew2"]).reshape(F, H)
    b1e = inputs["bmu1"] + np.exp(inputs["bls1"]) * inputs["eb1"]
    dve_feats = np.arange(2 * PASS2_ACT_PAIRS, F)
    lin_const = NEG * np.float32(
        (w2e[dve_feats].astype(np.float16).astype(np.float32)
         * b1e[dve_feats].astype(np.float32)).sum())
    yconst = np.float32(b2e.sum() + inputs["bias"][0] + lin_const)
    ys = []
    for c in range(NCORES):
        part = np.asarray(res.results[c]["y_part"])
        ys.append(part.reshape(BL) + yconst)
    y = np.concatenate(ys).astype(np.float32)
    kl = np.float32(np.asarray(res.results[0]["kl"]).reshape(()))
    return y, kl
